# revision 4
# baseline (speedup 1.0000x reference)
"""BatchAllTripletLoss on 8 Trainium2 NeuronCores via Bass/Tile.

Math: for anchors i, positives j (same label, j!=i), negatives k (diff label):
  total        = sum_{i,j,k} relu(d_ij - d_ik + margin)
  num_non_easy = #{(i,j,k): d_ik < d_ij + margin}
  loss         = total / num_non_easy ; frac = num_non_easy / num_valid

Sharding: anchors i split 80 per core. Per core the [80, 640] distance-row
block is computed with PE matmuls.  The O(n^3) part runs per anchor a:
the row v'_k = d_ak + BIG*(same label) is partition-broadcast to [128, 640];
thresholds t_j = d_aj + margin live transposed as per-partition scalars.
ACT does fused Relu(t - v') + accumulate (hinge sums); DVE does fused
is_lt + accumulate (counts).  Positive-pair masking happens on the [128, 80]
per-j sums at the end.  num_valid is pure label counting (host).
"""

import numpy as np

N = 640
D = 128
NCORES = 8
NLOC = N // NCORES            # 80 anchors per core
NCT = N // 128                # 5 j-tiles of 128
MARGIN = 1.9
BIG = 1.0e9

_CACHE = {}


def _build_program():
    import concourse.bass as bass
    import concourse.bacc as bacc
    import concourse.mybir as mybir
    import concourse.tile as tile
    from concourse.masks import make_identity

    f32 = mybir.dt.float32
    Alu = mybir.AluOpType
    Act = mybir.ActivationFunctionType

    nc = bacc.Bacc("TRN2", target_bir_lowering=False, debug=False,
                   num_devices=NCORES)

    efT = nc.declare_dram_parameter("efT", [D, N], f32, isOutput=False)
    elocT = nc.declare_dram_parameter("elocT", [D, NLOC], f32, isOutput=False)
    labrow = nc.declare_dram_parameter("labrow", [1, N], f32, isOutput=False)
    labT = nc.declare_dram_parameter("labT", [128, NCT], f32, isOutput=False)
    llocrow = nc.declare_dram_parameter("llocrow", [1, NLOC], f32, isOutput=False)
    llocT = nc.declare_dram_parameter("llocT", [NLOC, 1], f32, isOutput=False)
    eye = nc.declare_dram_parameter("eye", [128, NCT * NLOC], f32, isOutput=False)
    out_d = nc.declare_dram_parameter("out", [128, 2 * NCT], f32, isOutput=True)

    with tile.TileContext(nc) as tc:
        with (
            tc.tile_pool(name="singles", bufs=1) as sg,
            tc.tile_pool(name="vbp", bufs=4) as vbp,
            tc.tile_pool(name="stp", bufs=4) as stp,
            tc.tile_pool(name="scra", bufs=2) as scra,
            tc.tile_pool(name="scrd", bufs=2) as scrd,
            tc.tile_pool(name="ps_mm", bufs=1, space="PSUM") as ps_mm,
            tc.tile_pool(name="ps_tr", bufs=2, space="PSUM") as ps_tr,
        ):
            # ---- load inputs ----
            EF = sg.tile([D, N], f32)
            nc.gpsimd.dma_start(out=EF[:], in_=efT[:])
            EL = sg.tile([D, NLOC], f32)
            nc.gpsimd.dma_start(out=EL[:], in_=elocT[:])
            LR = sg.tile([1, N], f32)
            nc.gpsimd.dma_start(out=LR[:], in_=labrow[:])
            LT = sg.tile([128, NCT], f32)
            nc.gpsimd.dma_start(out=LT[:], in_=labT[:])
            LLR = sg.tile([1, NLOC], f32)
            nc.gpsimd.dma_start(out=LLR[:], in_=llocrow[:])
            LLT = sg.tile([NLOC, 1], f32)
            nc.gpsimd.dma_start(out=LLT[:], in_=llocT[:])
            EYE = sg.tile([128, NCT * NLOC], f32)
            nc.gpsimd.dma_start(out=EYE[:], in_=eye[:])

            ident = sg.tile([128, 128], f32)
            make_identity(nc, ident[:])
            ones = sg.tile([128, 1], f32)
            nc.vector.memset(ones[:], 1.0)

            # ---- pairwise distance rows for local anchors ----
            # sq_full[k] = ||e_k||^2 ; sq_loc[a] = ||e_(g0+a)||^2
            Esq = sg.tile([D, N], f32)
            nc.vector.tensor_mul(Esq[:], EF[:], EF[:])
            ELsq = sg.tile([D, NLOC], f32)
            nc.vector.tensor_mul(ELsq[:], EL[:], EL[:])

            sqf_ps = ps_mm.tile([1, N], f32, tag="sqf")
            nc.tensor.matmul(sqf_ps[:, 0:512], ones[:], Esq[:, 0:512])
            nc.tensor.matmul(sqf_ps[:, 512:N], ones[:], Esq[:, 512:N])
            SQF = sg.tile([1, N], f32)
            nc.vector.tensor_copy(SQF[:], sqf_ps[:])

            sql_ps = ps_mm.tile([NLOC, 1], f32, tag="sql")
            nc.tensor.matmul(sql_ps[:], ELsq[:], ones[:])
            SQL = sg.tile([NLOC, 1], f32)
            nc.vector.tensor_copy(SQL[:], sql_ps[:])

            dot_ps = ps_mm.tile([NLOC, N], f32, tag="dot")
            nc.tensor.matmul(dot_ps[:, 0:512], EL[:], EF[:, 0:512])
            nc.tensor.matmul(dot_ps[:, 512:N], EL[:], EF[:, 512:N])

            # pre = sq_loc - 2*dot + sq_full ; dist = sqrt(relu(pre))
            A = sg.tile([NLOC, N], f32)
            nc.vector.tensor_scalar(out=A[:], in0=dot_ps[:], scalar1=-2.0,
                                    scalar2=SQL[:], op0=Alu.mult, op1=Alu.add)
            SQB = sg.tile([128, N], f32)
            nc.gpsimd.partition_broadcast(SQB[0:NLOC, :], SQF[:], channels=NLOC)
            PRE = sg.tile([NLOC, N], f32)
            nc.vector.tensor_add(PRE[:], A[:], SQB[0:NLOC, :])
            nc.vector.tensor_scalar(out=PRE[:], in0=PRE[:], scalar1=0.0,
                                    scalar2=None, op0=Alu.max)
            DIST = sg.tile([NLOC, N], f32)
            nc.scalar.activation(out=DIST[:], in_=PRE[:], func=Act.Sqrt)

            # v'row = dist + BIG * (label equal, incl. diagonal)
            LBC = sg.tile([128, N], f32)
            nc.gpsimd.partition_broadcast(LBC[0:NLOC, :], LR[:], channels=NLOC)
            EQB = sg.tile([NLOC, N], f32)
            nc.vector.tensor_scalar(out=EQB[:], in0=LBC[0:NLOC, :], scalar1=LLT[:],
                                    scalar2=BIG, op0=Alu.is_equal, op1=Alu.mult)
            VROW = sg.tile([NLOC, N], f32)
            nc.vector.tensor_add(VROW[:], DIST[:], EQB[:])

            # thresholds transposed: tsb[c][p, a] = dist[a, c*128+p] + margin
            tsb = []
            for c in range(NCT):
                tr_ps = ps_tr.tile([128, NLOC], f32, tag="tr")
                nc.tensor.transpose(tr_ps[:], DIST[:, c * 128:(c + 1) * 128],
                                    ident[0:NLOC, 0:NLOC])
                t = sg.tile([128, NLOC], f32, tag=f"tsb{c}", name=f"tsb{c}")
                nc.vector.tensor_scalar_add(out=t[:], in0=tr_ps[:], scalar1=MARGIN)
                tsb.append(t)

            # positive mask transposed: (lab_j == lab_a) - eye
            LLB = sg.tile([128, NLOC], f32)
            nc.gpsimd.partition_broadcast(LLB[:], LLR[:], channels=128)
            posT = []
            for c in range(NCT):
                p = sg.tile([128, NLOC], f32, tag=f"posT{c}", name=f"posT{c}")
                nc.vector.tensor_scalar(out=p[:], in0=LLB[:], scalar1=LT[:, c:c + 1],
                                        scalar2=None, op0=Alu.is_equal)
                nc.vector.tensor_sub(p[:], p[:], EYE[:, c * NLOC:(c + 1) * NLOC])
                posT.append(p)

            # per-(j, anchor) accumulators
            S = [sg.tile([128, NLOC], f32, tag=f"S{c}", name=f"S{c}")
                 for c in range(NCT)]
            C = [sg.tile([128, NLOC], f32, tag=f"C{c}", name=f"C{c}")
                 for c in range(NCT)]

            # ---- main loop: one broadcast + 5 ACT + 5 DVE per anchor ----
            for a in range(NLOC):
                st = stp.tile([1, N], f32, tag="st")
                nc.sync.dma_start(out=st[:], in_=VROW[a:a + 1, :])
                vb = vbp.tile([128, N], f32, tag="vb")
                nc.gpsimd.partition_broadcast(vb[:], st[:], channels=128)
                for c in range(NCT):
                    sa = scra.tile([128, N], f32, tag="sa")
                    nc.scalar.activation(out=sa[:], in_=vb[:], func=Act.Relu,
                                         bias=tsb[c][:, a:a + 1], scale=-1.0,
                                         accum_out=S[c][:, a:a + 1])
                    sd = scrd.tile([128, N], f32, tag="sd")
                    nc.vector.tensor_scalar(out=sd[:], in0=vb[:],
                                            scalar1=tsb[c][:, a:a + 1],
                                            scalar2=None, op0=Alu.is_lt,
                                            op1=Alu.add,
                                            accum_out=C[c][:, a:a + 1])

            # ---- masked reduce over anchors ----
            OUTS = sg.tile([128, 2 * NCT], f32)
            for c in range(NCT):
                tmp = scrd.tile([128, NLOC], f32, tag="red")
                nc.vector.tensor_mul(tmp[:], S[c][:], posT[c][:])
                nc.vector.tensor_reduce(out=OUTS[:, c:c + 1], in_=tmp[:],
                                        axis=mybir.AxisListType.X, op=Alu.add)
                tmp2 = scrd.tile([128, NLOC], f32, tag="red")
                nc.vector.tensor_mul(tmp2[:], C[c][:], posT[c][:])
                nc.vector.tensor_reduce(out=OUTS[:, NCT + c:NCT + c + 1],
                                        in_=tmp2[:],
                                        axis=mybir.AxisListType.X, op=Alu.add)
            nc.gpsimd.dma_start(out=out_d[:], in_=OUTS[:])

    nc.compile()
    return nc


def _get_program():
    if "nc" not in _CACHE:
        _CACHE["nc"] = _build_program()
    return _CACHE["nc"]


def _make_inputs(embeddings: np.ndarray, labels: np.ndarray):
    e = np.ascontiguousarray(embeddings.reshape(N, D).astype(np.float32))
    lab = labels.reshape(N).astype(np.float32)
    efT = np.ascontiguousarray(e.T)                       # [D, N]
    labrow = lab.reshape(1, N)
    labT = np.ascontiguousarray(lab.reshape(NCT, 128).T)  # [128, NCT]

    in_maps = []
    for r in range(NCORES):
        g0 = r * NLOC
        eye = np.zeros((128, NCT * NLOC), np.float32)
        for a in range(NLOC):
            j = g0 + a
            eye[j % 128, (j // 128) * NLOC + a] = 1.0
        in_maps.append({
            "efT": efT,
            "elocT": np.ascontiguousarray(efT[:, g0:g0 + NLOC]),
            "labrow": labrow,
            "labT": labT,
            "llocrow": np.ascontiguousarray(lab[g0:g0 + NLOC].reshape(1, NLOC)),
            "llocT": np.ascontiguousarray(lab[g0:g0 + NLOC].reshape(NLOC, 1)),
            "eye": eye,
        })
    return in_maps


def run_on_device(embeddings: np.ndarray, labels: np.ndarray, **run_kwargs):
    from concourse.bass_utils import run_bass_kernel_spmd
    nc = _get_program()
    in_maps = _make_inputs(embeddings, labels)
    res = run_bass_kernel_spmd(nc, in_maps, core_ids=list(range(NCORES)),
                               **run_kwargs)
    total = 0.0
    count = 0.0
    for r in range(NCORES):
        o = res.results[r]["out"].astype(np.float64)
        total += o[:, 0:NCT].sum()
        count += o[:, NCT:2 * NCT].sum()
    return total, count, res


def kernel(embeddings: np.ndarray, labels: np.ndarray):
    total, count, _ = run_on_device(embeddings, labels)

    lab = np.asarray(labels).reshape(-1)
    cnt = np.bincount(lab.astype(np.int64), minlength=1)
    per = cnt[lab.astype(np.int64)]
    num_valid = int(((per - 1) * (N - per)).sum())

    nv = np.float32(num_valid)
    ne = np.float32(count)
    tot = np.float32(total)
    if ne > 0:
        loss = np.float32(tot / np.maximum(ne, np.float32(1.0)))
    else:
        loss = np.float32(0.0)
    frac = np.float32(ne / (nv + np.float32(1e-16)))
    return (np.array(loss, np.float32), np.array(nv, np.float32),
            np.array(ne, np.float32), np.array(frac, np.float32))


# revision 7
# speedup vs baseline: 1.5337x; 1.5337x over previous
"""BatchAllTripletLoss on 8 Trainium2 NeuronCores via Bass/Tile.

Math: for anchors i, positives j (same label, j!=i), negatives k (diff label):
  total        = sum_{i,j,k} relu(d_ij - d_ik + margin)
  num_non_easy = #{(i,j,k): d_ik < d_ij + margin}
  loss         = total / num_non_easy ; frac = num_non_easy / num_valid

Sharding: anchors i split 80 per core; each core computes its [80, 640]
distance-row block with PE matmuls.

O(n^3) strategy (per anchor a):
  - masked row v'_k = d_ak + BIG*(same label), bf16, partition-broadcast to
    [128, 640] (GPSIMD).
  - masked thresholds t'_j = (d_aj + margin) * positive_mask (0 when not a
    positive pair), kept f32 per-partition and split hi/lo into bf16.
  - DVE builds the 0/1 matrix M[j, k] = (v'_k < t'_j) in ONE bf16
    tensor_scalar (is_lt, no accum -> 4x mode, ~300ns per [128,640] tile).
  - PE reduces M with lhsT = [t'_hi | t'_lo | 1 | 0] (bf16):
       psum row base+0/1: sum_j t'_j * M[j,k]  (hi/lo parts)
       psum row base+2:   q[k] = sum_j M[j,k]
    accumulated over the 5 j-tiles; 3 anchors per psum tile (bases 0/32/64).
  - ACT free-sums psum rows (Identity + accum); DVE does the fused
    (q * dist) reduce for the  sum_k d_ak * q_ak  term.
  total = sum(t'*M) - sum(d*q);  count = sum(q).  Host combines in f64.
num_valid is pure label counting (host, exact).
"""

import numpy as np

N = 640
D = 128
NCORES = 8
NLOC = N // NCORES            # 80 anchors per core
NCT = N // 128                # 5 j-tiles of 128
NGRP = (NLOC + 2) // 3        # 27 psum groups, 3 anchors each (last has 2)
MARGIN = 1.9
BIG = 1.0e9

_CACHE = {}


def _build_program():
    import concourse.bass as bass
    import concourse.bacc as bacc
    import concourse.mybir as mybir
    import concourse.tile as tile
    from concourse.masks import make_identity

    f32 = mybir.dt.float32
    bf16 = mybir.dt.bfloat16
    Alu = mybir.AluOpType
    Act = mybir.ActivationFunctionType

    nc = bacc.Bacc("TRN2", target_bir_lowering=False, debug=False,
                   num_devices=NCORES)

    efT = nc.declare_dram_parameter("efT", [D, N], f32, isOutput=False)
    elocT = nc.declare_dram_parameter("elocT", [D, NLOC], f32, isOutput=False)
    labrow = nc.declare_dram_parameter("labrow", [1, N], f32, isOutput=False)
    labT = nc.declare_dram_parameter("labT", [128, NCT], f32, isOutput=False)
    llocrow = nc.declare_dram_parameter("llocrow", [1, NLOC], f32, isOutput=False)
    llocT = nc.declare_dram_parameter("llocT", [NLOC, 1], f32, isOutput=False)
    eye = nc.declare_dram_parameter("eye", [128, NCT * NLOC], f32, isOutput=False)
    # out: [128, 2*NGRP(wsums) + 2*NGRP(p2)] = [128, 108]
    out_d = nc.declare_dram_parameter("out", [128, 4 * NGRP], f32, isOutput=True)

    with tile.TileContext(nc) as tc:
        with (
            tc.tile_pool(name="singles", bufs=1) as sg,
            tc.tile_pool(name="vbp", bufs=4) as vbp,
            tc.tile_pool(name="stp", bufs=4) as stp,
            tc.tile_pool(name="mtp", bufs=6) as mtp,
            tc.tile_pool(name="dpp", bufs=2) as dpp,
            tc.tile_pool(name="drs", bufs=3) as drs,
            tc.tile_pool(name="ps_mm", bufs=1, space="PSUM") as ps_mm,
            tc.tile_pool(name="ps_tr", bufs=1, space="PSUM") as ps_tr,
            tc.tile_pool(name="ps_wq1", bufs=2, space="PSUM") as ps_wq1,
            tc.tile_pool(name="ps_wq2", bufs=2, space="PSUM") as ps_wq2,
        ):
            # ---- load inputs ----
            EF = sg.tile([D, N], f32)
            nc.gpsimd.dma_start(out=EF[:], in_=efT[:])
            EL = sg.tile([D, NLOC], f32)
            nc.gpsimd.dma_start(out=EL[:], in_=elocT[:])
            LR = sg.tile([1, N], f32)
            nc.gpsimd.dma_start(out=LR[:], in_=labrow[:])
            LT = sg.tile([128, NCT], f32)
            nc.gpsimd.dma_start(out=LT[:], in_=labT[:])
            LLR = sg.tile([1, NLOC], f32)
            nc.gpsimd.dma_start(out=LLR[:], in_=llocrow[:])
            LLT = sg.tile([NLOC, 1], f32)
            nc.gpsimd.dma_start(out=LLT[:], in_=llocT[:])
            EYE = sg.tile([128, NCT * NLOC], f32)
            nc.gpsimd.dma_start(out=EYE[:], in_=eye[:])

            ident = sg.tile([128, 128], f32)
            make_identity(nc, ident[:])
            ones = sg.tile([128, 1], f32)
            nc.vector.memset(ones[:], 1.0)

            # ---- pairwise distance rows for local anchors ----
            Esq = sg.tile([D, N], f32)
            nc.vector.tensor_mul(Esq[:], EF[:], EF[:])
            ELsq = sg.tile([D, NLOC], f32)
            nc.vector.tensor_mul(ELsq[:], EL[:], EL[:])

            sqf_ps = ps_mm.tile([1, N], f32, tag="pro", name="sqf")
            nc.tensor.matmul(sqf_ps[:, 0:512], ones[:], Esq[:, 0:512])
            nc.tensor.matmul(sqf_ps[:, 512:N], ones[:], Esq[:, 512:N])
            SQF = sg.tile([1, N], f32)
            nc.vector.tensor_copy(SQF[:], sqf_ps[:])

            sql_ps = ps_mm.tile([NLOC, 1], f32, tag="pro", name="sql")
            nc.tensor.matmul(sql_ps[:], ELsq[:], ones[:])
            SQL = sg.tile([NLOC, 1], f32)
            nc.vector.tensor_copy(SQL[:], sql_ps[:])

            dot_ps = ps_mm.tile([NLOC, N], f32, tag="pro", name="dot")
            nc.tensor.matmul(dot_ps[:, 0:512], EL[:], EF[:, 0:512])
            nc.tensor.matmul(dot_ps[:, 512:N], EL[:], EF[:, 512:N])

            A = sg.tile([NLOC, N], f32)
            nc.vector.tensor_scalar(out=A[:], in0=dot_ps[:], scalar1=-2.0,
                                    scalar2=SQL[:], op0=Alu.mult, op1=Alu.add)
            SQB = sg.tile([128, N], f32)
            nc.gpsimd.partition_broadcast(SQB[0:NLOC, :], SQF[:], channels=NLOC)
            PRE = sg.tile([NLOC, N], f32)
            nc.vector.tensor_add(PRE[:], A[:], SQB[0:NLOC, :])
            nc.vector.tensor_scalar(out=PRE[:], in0=PRE[:], scalar1=0.0,
                                    scalar2=None, op0=Alu.max)
            DIST = sg.tile([NLOC, N], f32)
            nc.scalar.activation(out=DIST[:], in_=PRE[:], func=Act.Sqrt)

            # masked v' row, bf16
            LBC = sg.tile([128, N], f32)
            nc.gpsimd.partition_broadcast(LBC[0:NLOC, :], LR[:], channels=NLOC)
            EQB = sg.tile([NLOC, N], f32)
            nc.vector.tensor_scalar(out=EQB[:], in0=LBC[0:NLOC, :], scalar1=LLT[:],
                                    scalar2=BIG, op0=Alu.is_equal, op1=Alu.mult)
            VM = sg.tile([NLOC, N], f32)
            nc.vector.tensor_add(VM[:], DIST[:], EQB[:])
            VMB = sg.tile([NLOC, N], bf16)
            nc.vector.tensor_copy(VMB[:], VM[:])

            # positive mask transposed: (lab_j == lab_a) - eye
            LLB = sg.tile([128, NLOC], f32)
            nc.gpsimd.partition_broadcast(LLB[:], LLR[:], channels=128)
            posT = []
            for c in range(NCT):
                p = sg.tile([128, NLOC], f32, tag=f"posT{c}", name=f"posT{c}")
                nc.vector.tensor_scalar(out=p[:], in0=LLB[:], scalar1=LT[:, c:c + 1],
                                        scalar2=None, op0=Alu.is_equal)
                nc.vector.tensor_sub(p[:], p[:], EYE[:, c * NLOC:(c + 1) * NLOC])
                posT.append(p)

            # thresholds: tp[c][p, a] = (dist[a, c*128+p] + margin) * posT
            # plus bf16 hi/lo split packed into lhsT tiles [128, NLOC, 4]
            tp = []
            lhsb = []
            for c in range(NCT):
                tr_ps = ps_tr.tile([128, NLOC], f32, tag="tr")
                nc.tensor.transpose(tr_ps[:], DIST[:, c * 128:(c + 1) * 128],
                                    ident[0:NLOC, 0:NLOC])
                t = sg.tile([128, NLOC], f32, tag=f"tp{c}", name=f"tp{c}")
                nc.vector.tensor_scalar_add(out=t[:], in0=tr_ps[:], scalar1=MARGIN)
                nc.vector.tensor_mul(t[:], t[:], posT[c][:])
                tp.append(t)

                L = sg.tile([128, NLOC, 4], bf16, tag=f"lhsb{c}", name=f"lhsb{c}")
                nc.vector.memset(L[:], 0.0)
                nc.vector.tensor_copy(L[:, :, 0], t[:])            # t_hi (bf16)
                thf = sg.tile([128, NLOC], f32, tag="thf", name="thf")
                nc.vector.tensor_copy(thf[:], L[:, :, 0])          # back to f32
                nc.vector.tensor_sub(thf[:], t[:], thf[:])         # t_lo
                nc.vector.tensor_copy(L[:, :, 1], thf[:])
                nc.vector.memset(L[:, :, 2], 1.0)
                lhsb.append(L)

            # ---- main loop ----
            dr_tiles = []   # (DR accum tile [128, 2], P2 accum tile [128, 2])
            for g in range(NGRP):
                na = min(3, NLOC - 3 * g)
                wq1 = ps_wq1.tile([128, 512], f32, tag="wq1", name="wq1")
                wq2 = ps_wq2.tile([128, 128], f32, tag="wq2", name="wq2")
                dp = dpp.tile([128, N], f32, tag="dp", name="dp")
                for m in range(na):
                    a = 3 * g + m
                    base = 32 * m
                    st = stp.tile([1, N], bf16, tag="st", name="st")
                    nc.sync.dma_start(out=st[:], in_=VMB[a:a + 1, :])
                    vb = vbp.tile([128, N], bf16, tag="vb", name="vb")
                    nc.gpsimd.partition_broadcast(vb[:], st[:], channels=128)
                    nc.sync.dma_start(out=dp[base + 2:base + 3, :],
                                      in_=DIST[a:a + 1, :])
                    for c in range(NCT):
                        mt = mtp.tile([128, N], bf16, tag="mt", name="mt")
                        nc.vector.tensor_scalar(out=mt[:], in0=vb[:],
                                                scalar1=tp[c][:, a:a + 1],
                                                scalar2=None, op0=Alu.is_lt)
                        nc.tensor.matmul(wq1[base:base + 4, :],
                                         lhsb[c][:, a], mt[:, 0:512],
                                         start=(c == 0), stop=(c == NCT - 1))
                        nc.tensor.matmul(wq2[base:base + 4, :],
                                         lhsb[c][:, a], mt[:, 512:N],
                                         start=(c == 0), stop=(c == NCT - 1))
                # drain group: ACT free-sums all psum rows; DVE fused q*dist
                DR = drs.tile([128, 2], f32, tag="dr", name="dr")
                P2 = drs.tile([128, 2], f32, tag="p2", name="p2")
                sa1 = drs.tile([128, 512], f32, tag="sa1", name="sa1")
                sa2 = drs.tile([128, 128], f32, tag="sa2", name="sa2")
                sb1 = drs.tile([128, 512], f32, tag="sb1", name="sb1")
                sb2 = drs.tile([128, 128], f32, tag="sb2", name="sb2")
                nc.scalar.activation(out=sa1[:], in_=wq1[:], func=Act.Identity,
                                     bias=0.0, scale=1.0, accum_out=DR[:, 0:1])
                nc.scalar.activation(out=sa2[:], in_=wq2[:], func=Act.Identity,
                                     bias=0.0, scale=1.0, accum_out=DR[:, 1:2])
                nc.vector.scalar_tensor_tensor(out=sb1[:], in0=wq1[:],
                                               scalar=1.0, in1=dp[:, 0:512],
                                               op0=Alu.mult, op1=Alu.mult,
                                               accum_out=P2[:, 0:1])
                nc.vector.scalar_tensor_tensor(out=sb2[:], in0=wq2[:],
                                               scalar=1.0, in1=dp[:, 512:N],
                                               op0=Alu.mult, op1=Alu.mult,
                                               accum_out=P2[:, 1:2])
                dr_tiles.append((DR, P2))

            # ---- stage outputs ----
            OUTS = sg.tile([128, 4 * NGRP], f32)
            for g, (DR, P2) in enumerate(dr_tiles):
                nc.vector.tensor_copy(OUTS[:, 2 * g:2 * g + 2], DR[:])
                nc.vector.tensor_copy(OUTS[:, 2 * NGRP + 2 * g:2 * NGRP + 2 * g + 2],
                                      P2[:])
            nc.gpsimd.dma_start(out=out_d[:], in_=OUTS[:])

    nc.compile()
    return nc


def _get_program():
    if "nc" not in _CACHE:
        _CACHE["nc"] = _build_program()
    return _CACHE["nc"]


def _make_inputs(embeddings: np.ndarray, labels: np.ndarray):
    e = np.ascontiguousarray(embeddings.reshape(N, D).astype(np.float32))
    lab = labels.reshape(N).astype(np.float32)
    efT = np.ascontiguousarray(e.T)                       # [D, N]
    labrow = lab.reshape(1, N)
    labT = np.ascontiguousarray(lab.reshape(NCT, 128).T)  # [128, NCT]

    in_maps = []
    for r in range(NCORES):
        g0 = r * NLOC
        eye = np.zeros((128, NCT * NLOC), np.float32)
        for a in range(NLOC):
            j = g0 + a
            eye[j % 128, (j // 128) * NLOC + a] = 1.0
        in_maps.append({
            "efT": efT,
            "elocT": np.ascontiguousarray(efT[:, g0:g0 + NLOC]),
            "labrow": labrow,
            "labT": labT,
            "llocrow": np.ascontiguousarray(lab[g0:g0 + NLOC].reshape(1, NLOC)),
            "llocT": np.ascontiguousarray(lab[g0:g0 + NLOC].reshape(NLOC, 1)),
            "eye": eye,
        })
    return in_maps


def run_on_device(embeddings: np.ndarray, labels: np.ndarray, **run_kwargs):
    from concourse.bass_utils import run_bass_kernel_spmd
    nc = _get_program()
    in_maps = _make_inputs(embeddings, labels)
    res = run_bass_kernel_spmd(nc, in_maps, core_ids=list(range(NCORES)),
                               **run_kwargs)
    total = 0.0
    count = 0.0
    for r in range(NCORES):
        o = res.results[r]["out"].astype(np.float64)
        for g in range(NGRP):
            na = min(3, NLOC - 3 * g)
            for m in range(na):
                base = 32 * m
                for ch in range(2):
                    w_hi = o[base + 0, 2 * g + ch]
                    w_lo = o[base + 1, 2 * g + ch]
                    q = o[base + 2, 2 * g + ch]
                    p2 = o[base + 2, 2 * NGRP + 2 * g + ch]
                    total += (w_hi + w_lo) - p2
                    count += q
    return total, count, res


def kernel(embeddings: np.ndarray, labels: np.ndarray):
    total, count, _ = run_on_device(embeddings, labels)

    lab = np.asarray(labels).reshape(-1)
    cnt = np.bincount(lab.astype(np.int64), minlength=1)
    per = cnt[lab.astype(np.int64)]
    num_valid = int(((per - 1) * (N - per)).sum())

    nv = np.float32(num_valid)
    ne = np.float32(count)
    tot = np.float32(total)
    if ne > 0:
        loss = np.float32(tot / np.maximum(ne, np.float32(1.0)))
    else:
        loss = np.float32(0.0)
    frac = np.float32(ne / (nv + np.float32(1e-16)))
    return (np.array(loss, np.float32), np.array(nv, np.float32),
            np.array(ne, np.float32), np.array(frac, np.float32))


# revision 8
# speedup vs baseline: 2.1461x; 1.3993x over previous
"""BatchAllTripletLoss on 8 Trainium2 NeuronCores via Bass/Tile.

Math: for anchors i, positives j (same label, j!=i), negatives k (diff label):
  total        = sum_{i,j,k} relu(d_ij - d_ik + margin)
  num_non_easy = #{(i,j,k): d_ik < d_ij + margin}
  loss         = total / num_non_easy ; frac = num_non_easy / num_valid

Sharding: anchors i split 80 per core; each core computes its [80, 640]
distance-row block with PE matmuls.

O(n^3) strategy (per anchor a):
  - masked row v'_k = d_ak + BIG*(same label), bf16, partition-broadcast to
    [128, 640] (GPSIMD).
  - masked thresholds t'_j = (d_aj + margin) * positive_mask (0 when not a
    positive pair), kept f32 per-partition and split hi/lo into bf16.
  - DVE builds the 0/1 matrix M[j, k] = (v'_k < t'_j) in ONE bf16
    tensor_scalar (is_lt, no accum -> 4x mode, ~300ns per [128,640] tile).
  - PE reduces M with lhsT = [t'_hi | t'_lo | 1 | 0] (bf16):
       psum row base+0/1: sum_j t'_j * M[j,k]  (hi/lo parts)
       psum row base+2:   q[k] = sum_j M[j,k]
    accumulated over the 5 j-tiles; 3 anchors per psum tile (bases 0/32/64).
  - ACT free-sums psum rows (Identity + accum); DVE does the fused
    (q * dist) reduce for the  sum_k d_ak * q_ak  term.
  total = sum(t'*M) - sum(d*q);  count = sum(q).  Host combines in f64.
num_valid is pure label counting (host, exact).
"""

import numpy as np

N = 640
D = 128
NCORES = 8
NLOC = N // NCORES            # 80 anchors per core
NCT = N // 128                # 5 j-tiles of 128
NGRP = (NLOC + 2) // 3        # 27 psum groups, 3 anchors each (last has 2)
MARGIN = 1.9
BIG = 1.0e9

_CACHE = {}


def _build_program():
    import concourse.bass as bass
    import concourse.bacc as bacc
    import concourse.mybir as mybir
    import concourse.tile as tile
    from concourse.masks import make_identity

    f32 = mybir.dt.float32
    bf16 = mybir.dt.bfloat16
    Alu = mybir.AluOpType
    Act = mybir.ActivationFunctionType

    nc = bacc.Bacc("TRN2", target_bir_lowering=False, debug=False,
                   num_devices=NCORES)

    efT = nc.declare_dram_parameter("efT", [D, N], f32, isOutput=False)
    elocT = nc.declare_dram_parameter("elocT", [D, NLOC], f32, isOutput=False)
    labrow = nc.declare_dram_parameter("labrow", [1, N], f32, isOutput=False)
    labT = nc.declare_dram_parameter("labT", [128, NCT], f32, isOutput=False)
    llocrow = nc.declare_dram_parameter("llocrow", [1, NLOC], f32, isOutput=False)
    llocT = nc.declare_dram_parameter("llocT", [NLOC, 1], f32, isOutput=False)
    eye = nc.declare_dram_parameter("eye", [128, NCT * NLOC], f32, isOutput=False)
    # out: [128, 2*NGRP(wsums) + 2*NGRP(p2)] = [128, 108]
    out_d = nc.declare_dram_parameter("out", [128, 4 * NGRP], f32, isOutput=True)

    with tile.TileContext(nc) as tc:
        with (
            tc.tile_pool(name="singles", bufs=1) as sg,
            tc.tile_pool(name="vbp", bufs=4) as vbp,
            tc.tile_pool(name="stp", bufs=4) as stp,
            tc.tile_pool(name="mtp", bufs=6) as mtp,
            tc.tile_pool(name="dpp", bufs=2) as dpp,
            tc.tile_pool(name="drs", bufs=3) as drs,
            tc.tile_pool(name="dram", bufs=1, space="DRAM") as dram,
            tc.tile_pool(name="ps_mm", bufs=1, space="PSUM") as ps_mm,
            tc.tile_pool(name="ps_tr", bufs=1, space="PSUM") as ps_tr,
            tc.tile_pool(name="ps_wq1", bufs=2, space="PSUM") as ps_wq1,
            tc.tile_pool(name="ps_wq2", bufs=2, space="PSUM") as ps_wq2,
        ):
            # ---- load inputs ----
            EF = sg.tile([D, N], f32)
            nc.gpsimd.dma_start(out=EF[:], in_=efT[:])
            EL = sg.tile([D, NLOC], f32)
            nc.gpsimd.dma_start(out=EL[:], in_=elocT[:])
            LR = sg.tile([1, N], f32)
            nc.gpsimd.dma_start(out=LR[:], in_=labrow[:])
            LT = sg.tile([128, NCT], f32)
            nc.gpsimd.dma_start(out=LT[:], in_=labT[:])
            LLR = sg.tile([1, NLOC], f32)
            nc.gpsimd.dma_start(out=LLR[:], in_=llocrow[:])
            LLT = sg.tile([NLOC, 1], f32)
            nc.gpsimd.dma_start(out=LLT[:], in_=llocT[:])
            EYE = sg.tile([128, NCT * NLOC], f32)
            nc.gpsimd.dma_start(out=EYE[:], in_=eye[:])

            ident = sg.tile([128, 128], f32)
            make_identity(nc, ident[:])
            ones = sg.tile([128, 1], f32)
            nc.vector.memset(ones[:], 1.0)

            # ---- pairwise distance rows for local anchors ----
            Esq = sg.tile([D, N], f32)
            nc.vector.tensor_mul(Esq[:], EF[:], EF[:])
            ELsq = sg.tile([D, NLOC], f32)
            nc.vector.tensor_mul(ELsq[:], EL[:], EL[:])

            sqf_ps = ps_mm.tile([1, N], f32, tag="pro", name="sqf")
            nc.tensor.matmul(sqf_ps[:, 0:512], ones[:], Esq[:, 0:512])
            nc.tensor.matmul(sqf_ps[:, 512:N], ones[:], Esq[:, 512:N])
            SQF = sg.tile([1, N], f32)
            nc.vector.tensor_copy(SQF[:], sqf_ps[:])

            sql_ps = ps_mm.tile([NLOC, 1], f32, tag="pro", name="sql")
            nc.tensor.matmul(sql_ps[:], ELsq[:], ones[:])
            SQL = sg.tile([NLOC, 1], f32)
            nc.vector.tensor_copy(SQL[:], sql_ps[:])

            dot_ps = ps_mm.tile([NLOC, N], f32, tag="pro", name="dot")
            nc.tensor.matmul(dot_ps[:, 0:512], EL[:], EF[:, 0:512])
            nc.tensor.matmul(dot_ps[:, 512:N], EL[:], EF[:, 512:N])

            A = sg.tile([NLOC, N], f32)
            nc.vector.tensor_scalar(out=A[:], in0=dot_ps[:], scalar1=-2.0,
                                    scalar2=SQL[:], op0=Alu.mult, op1=Alu.add)
            sqf_d = dram.tile([1, N], f32)
            nc.sync.dma_start(out=sqf_d[:], in_=SQF[:])
            SQB = sg.tile([128, N], f32)
            nc.sync.dma_start(out=SQB[0:NLOC, :],
                              in_=sqf_d[:].to_broadcast([NLOC, N]))
            PRE = sg.tile([NLOC, N], f32)
            nc.vector.tensor_add(PRE[:], A[:], SQB[0:NLOC, :])
            nc.vector.tensor_scalar(out=PRE[:], in0=PRE[:], scalar1=0.0,
                                    scalar2=None, op0=Alu.max)
            DIST = sg.tile([NLOC, N], f32)
            nc.scalar.activation(out=DIST[:], in_=PRE[:], func=Act.Sqrt)

            # masked v' row, bf16
            LBC = sg.tile([128, N], f32)
            nc.sync.dma_start(out=LBC[0:NLOC, :],
                              in_=labrow[:].to_broadcast([NLOC, N]))
            EQB = sg.tile([NLOC, N], f32)
            nc.vector.tensor_scalar(out=EQB[:], in0=LBC[0:NLOC, :], scalar1=LLT[:],
                                    scalar2=BIG, op0=Alu.is_equal, op1=Alu.mult)
            VM = sg.tile([NLOC, N], f32)
            nc.vector.tensor_add(VM[:], DIST[:], EQB[:])
            VMB = sg.tile([NLOC, N], bf16)
            nc.vector.tensor_copy(VMB[:], VM[:])
            vmd = dram.tile([NLOC, N], bf16)
            nc.sync.dma_start(out=vmd[:], in_=VMB[:])

            # positive mask transposed: (lab_j == lab_a) - eye
            LLB = sg.tile([128, NLOC], f32)
            nc.sync.dma_start(out=LLB[:],
                              in_=llocrow[:].to_broadcast([128, NLOC]))
            posT = []
            for c in range(NCT):
                p = sg.tile([128, NLOC], f32, tag=f"posT{c}", name=f"posT{c}")
                nc.vector.tensor_scalar(out=p[:], in0=LLB[:], scalar1=LT[:, c:c + 1],
                                        scalar2=None, op0=Alu.is_equal)
                nc.vector.tensor_sub(p[:], p[:], EYE[:, c * NLOC:(c + 1) * NLOC])
                posT.append(p)

            # thresholds: tp[c][p, a] = (dist[a, c*128+p] + margin) * posT
            # plus bf16 hi/lo split packed into lhsT tiles [128, NLOC, 4]
            tp = []
            lhsb = []
            for c in range(NCT):
                tr_ps = ps_tr.tile([128, NLOC], f32, tag="tr")
                nc.tensor.transpose(tr_ps[:], DIST[:, c * 128:(c + 1) * 128],
                                    ident[0:NLOC, 0:NLOC])
                t = sg.tile([128, NLOC], f32, tag=f"tp{c}", name=f"tp{c}")
                nc.vector.tensor_scalar_add(out=t[:], in0=tr_ps[:], scalar1=MARGIN)
                nc.vector.tensor_mul(t[:], t[:], posT[c][:])
                tp.append(t)

                L = sg.tile([128, NLOC, 4], bf16, tag=f"lhsb{c}", name=f"lhsb{c}")
                nc.vector.memset(L[:], 0.0)
                nc.vector.tensor_copy(L[:, :, 0], t[:])            # t_hi (bf16)
                thf = sg.tile([128, NLOC], f32, tag="thf", name="thf")
                nc.vector.tensor_copy(thf[:], L[:, :, 0])          # back to f32
                nc.vector.tensor_sub(thf[:], t[:], thf[:])         # t_lo
                nc.vector.tensor_copy(L[:, :, 1], thf[:])
                nc.vector.memset(L[:, :, 2], 1.0)
                lhsb.append(L)

            # ---- main loop ----
            dr_tiles = []   # (DR accum tile [128, 2], P2 accum tile [128, 2])
            for g in range(NGRP):
                na = min(3, NLOC - 3 * g)
                wq1 = ps_wq1.tile([128, 512], f32, tag="wq1", name="wq1")
                wq2 = ps_wq2.tile([128, 128], f32, tag="wq2", name="wq2")
                dp = dpp.tile([128, N], f32, tag="dp", name="dp")
                for m in range(na):
                    a = 3 * g + m
                    base = 32 * m
                    vb = vbp.tile([128, N], bf16, tag="vb", name="vb")
                    nc.sync.dma_start(out=vb[:],
                                      in_=vmd[a:a + 1, :].to_broadcast([128, N]))
                    nc.sync.dma_start(out=dp[base + 2:base + 3, :],
                                      in_=DIST[a:a + 1, :])
                    for c in range(NCT):
                        mt = mtp.tile([128, N], bf16, tag="mt", name="mt")
                        nc.vector.tensor_scalar(out=mt[:], in0=vb[:],
                                                scalar1=tp[c][:, a:a + 1],
                                                scalar2=None, op0=Alu.is_lt)
                        nc.tensor.matmul(wq1[base:base + 4, :],
                                         lhsb[c][:, a], mt[:, 0:512],
                                         start=(c == 0), stop=(c == NCT - 1))
                        nc.tensor.matmul(wq2[base:base + 4, :],
                                         lhsb[c][:, a], mt[:, 512:N],
                                         start=(c == 0), stop=(c == NCT - 1))
                # drain group: ACT free-sums all psum rows; DVE fused q*dist
                DR = drs.tile([128, 2], f32, tag="dr", name="dr")
                P2 = drs.tile([128, 2], f32, tag="p2", name="p2")
                sa1 = drs.tile([128, 512], f32, tag="sa1", name="sa1")
                sa2 = drs.tile([128, 128], f32, tag="sa2", name="sa2")
                sb1 = drs.tile([128, 512], f32, tag="sb1", name="sb1")
                sb2 = drs.tile([128, 128], f32, tag="sb2", name="sb2")
                nc.scalar.activation(out=sa1[:], in_=wq1[:], func=Act.Identity,
                                     bias=0.0, scale=1.0, accum_out=DR[:, 0:1])
                nc.scalar.activation(out=sa2[:], in_=wq2[:], func=Act.Identity,
                                     bias=0.0, scale=1.0, accum_out=DR[:, 1:2])
                nc.vector.scalar_tensor_tensor(out=sb1[:], in0=wq1[:],
                                               scalar=1.0, in1=dp[:, 0:512],
                                               op0=Alu.mult, op1=Alu.mult,
                                               accum_out=P2[:, 0:1])
                nc.vector.scalar_tensor_tensor(out=sb2[:], in0=wq2[:],
                                               scalar=1.0, in1=dp[:, 512:N],
                                               op0=Alu.mult, op1=Alu.mult,
                                               accum_out=P2[:, 1:2])
                dr_tiles.append((DR, P2))

            # ---- stage outputs ----
            OUTS = sg.tile([128, 4 * NGRP], f32)
            for g, (DR, P2) in enumerate(dr_tiles):
                nc.vector.tensor_copy(OUTS[:, 2 * g:2 * g + 2], DR[:])
                nc.vector.tensor_copy(OUTS[:, 2 * NGRP + 2 * g:2 * NGRP + 2 * g + 2],
                                      P2[:])
            nc.gpsimd.dma_start(out=out_d[:], in_=OUTS[:])

    nc.compile()
    return nc


def _get_program():
    if "nc" not in _CACHE:
        _CACHE["nc"] = _build_program()
    return _CACHE["nc"]


def _make_inputs(embeddings: np.ndarray, labels: np.ndarray):
    e = np.ascontiguousarray(embeddings.reshape(N, D).astype(np.float32))
    lab = labels.reshape(N).astype(np.float32)
    efT = np.ascontiguousarray(e.T)                       # [D, N]
    labrow = lab.reshape(1, N)
    labT = np.ascontiguousarray(lab.reshape(NCT, 128).T)  # [128, NCT]

    in_maps = []
    for r in range(NCORES):
        g0 = r * NLOC
        eye = np.zeros((128, NCT * NLOC), np.float32)
        for a in range(NLOC):
            j = g0 + a
            eye[j % 128, (j // 128) * NLOC + a] = 1.0
        in_maps.append({
            "efT": efT,
            "elocT": np.ascontiguousarray(efT[:, g0:g0 + NLOC]),
            "labrow": labrow,
            "labT": labT,
            "llocrow": np.ascontiguousarray(lab[g0:g0 + NLOC].reshape(1, NLOC)),
            "llocT": np.ascontiguousarray(lab[g0:g0 + NLOC].reshape(NLOC, 1)),
            "eye": eye,
        })
    return in_maps


def run_on_device(embeddings: np.ndarray, labels: np.ndarray, **run_kwargs):
    from concourse.bass_utils import run_bass_kernel_spmd
    nc = _get_program()
    in_maps = _make_inputs(embeddings, labels)
    res = run_bass_kernel_spmd(nc, in_maps, core_ids=list(range(NCORES)),
                               **run_kwargs)
    total = 0.0
    count = 0.0
    for r in range(NCORES):
        o = res.results[r]["out"].astype(np.float64)
        for g in range(NGRP):
            na = min(3, NLOC - 3 * g)
            for m in range(na):
                base = 32 * m
                for ch in range(2):
                    w_hi = o[base + 0, 2 * g + ch]
                    w_lo = o[base + 1, 2 * g + ch]
                    q = o[base + 2, 2 * g + ch]
                    p2 = o[base + 2, 2 * NGRP + 2 * g + ch]
                    total += (w_hi + w_lo) - p2
                    count += q
    return total, count, res


def kernel(embeddings: np.ndarray, labels: np.ndarray):
    total, count, _ = run_on_device(embeddings, labels)

    lab = np.asarray(labels).reshape(-1)
    cnt = np.bincount(lab.astype(np.int64), minlength=1)
    per = cnt[lab.astype(np.int64)]
    num_valid = int(((per - 1) * (N - per)).sum())

    nv = np.float32(num_valid)
    ne = np.float32(count)
    tot = np.float32(total)
    if ne > 0:
        loss = np.float32(tot / np.maximum(ne, np.float32(1.0)))
    else:
        loss = np.float32(0.0)
    frac = np.float32(ne / (nv + np.float32(1e-16)))
    return (np.array(loss, np.float32), np.array(nv, np.float32),
            np.array(ne, np.float32), np.array(frac, np.float32))
